# revision 20
# baseline (speedup 1.0000x reference)
"""Trainium2 Bass kernel for nn_ConvDY2d (dynamic-weight 3x3 conv, CondConv-style).

Reference computation (B=16, C=O=256, H=W=64, K=4 mixing kernels):
  attn  = softmax(MLP(global_avg_pool(x)) / 30)            # [B, 4]
  w_mix = einsum('bk,koihw->boihw', attn, w_dyn)           # per-sample 3x3 conv kernel
  out[b] = conv2d(x[b], w_mix[b], padding=1)

Strategy: data-parallel over batch, 2 samples per NeuronCore across 8 cores.
Per core, the conv is an implicit GEMM: for each (out-channel block, 8-row
group) a [128, 512] PSUM tile accumulates 18 matmuls (2 c-blocks x 9 taps)
whose rhs are contiguous 512-element slices of a row-padded input image
([128c, 4226]).  Column wrap-around at row edges is fixed up afterwards by
subtracting border corrections computed with 12 strided-rhs matmuls per
output block.

Startup-latency schedule (the modeled DMA pipe is the whole game: packets
flow at ~230GB/s effective and a DMA's data is only "visible" ~1us after
its last packet, so everything is ordered around byte arrival):
  - x[b0] first as one 1MB DMA per c-block; pooling runs as two half-image
    partials per c-block, split across DVE and ACT.
  - MLP consts go on the DVE DMA queue so they bypass the bulk stream.
  - wdyn is loaded in (dy-row, c-block) interleaved order and the conv
    accumulation passes consume in exactly that order, so the matmul
    stream chases the arrivals with ~1us of margin everywhere.
  - weight mixing uses tensor_scalar (4x DVE mode) + tensor_tensor (2x)
    trees in bf16; the very first chunk is mixed per-tap ([128,256]) to
    cut first-matmul latency.
"""

import sys

if "/opt/trn_rl_repo" not in sys.path:
    sys.path.insert(0, "/opt/trn_rl_repo")

import numpy as np

B, C, H, W = 16, 256, 64, 64
O, K, KS = 256, 4, 3
MID = C // 4
INV_DELTA = 1.0 / 30.0
NCORES = 8
NB = B // NCORES            # samples per core
NPOS = KS * KS              # 9 taps
FPAD = 1 + 66 * W + 1       # padded image free size: 4226
ROW0 = 65                   # flat offset of input row 0 (= 1 + 1*64)

_CACHE = {}


def _build_nc():
    import concourse.bacc as bacc
    import concourse.tile as tile
    from concourse import mybir
    from concourse.tile_rust import add_dep_helper

    f32 = mybir.dt.float32
    bf16 = mybir.dt.bfloat16
    AX = mybir.AxisListType
    ALU = mybir.AluOpType
    ACTF = mybir.ActivationFunctionType

    nc = bacc.Bacc(target_bir_lowering=False, debug=False)

    x_d = nc.dram_tensor("x", [NB, C, H, W], bf16, kind="ExternalInput").ap()
    wd_d = nc.dram_tensor("wdynT", [KS, 2, 128, K * KS * O], bf16, kind="ExternalInput").ap()
    mlpc_d = nc.dram_tensor("mlpc", [128, 2 * MID + K + MID], f32, kind="ExternalInput").ap()
    out_d = nc.dram_tensor("out", [NB, O, H, W], f32, kind="ExternalOutput").ap()

    # conv accumulation passes in wdyn-arrival order
    PASSES = [(0, 0), (1, 0), (0, 1), (1, 1), (0, 2), (1, 2)]  # (cb, dy)

    with tile.TileContext(nc) as tc:
        with (
            tc.tile_pool(name="consts", bufs=1) as constp,
            tc.tile_pool(name="wdyn", bufs=1) as wdynp,
            tc.tile_pool(name="wmix", bufs=1) as wmixp,
            tc.tile_pool(name="xpad", bufs=1) as xpadp,
            tc.tile_pool(name="osb", bufs=6) as osbp,
            tc.tile_pool(name="convps", bufs=6, space="PSUM") as convps,
            tc.tile_pool(name="corrps", bufs=2, space="PSUM") as corrps,
        ):
            smallps = convps  # MLP psum shares the conv bank rotation
            ones_sb = constp.tile([1, 128], f32, tag="ones", name="ones_sb")
            nc.gpsimd.memset(ones_sb, 1.0)
            act_dummy = constp.tile([128, 32 * W], bf16, tag="actdum", name="act_dummy")

            # xpad tiles + pad memsets for both samples up front (gpsimd idle)
            xpad = [[None, None] for _ in range(NB)]
            for b in range(NB):
                for cb in range(2):
                    t = xpadp.tile([128, FPAD], bf16, tag=f"xpad{b}{cb}", name=f"xpad{b}{cb}")
                    nc.gpsimd.memset(t[:, 0:ROW0], 0.0)
                    nc.gpsimd.memset(t[:, ROW0 + H * W : FPAD], 0.0)
                    xpad[b][cb] = t

            def load_x(b):
                # one DMA per c-block: 128 descriptors x 8KB, minimal
                # packet/semaphore overhead in the modeled DMA pipe
                for cb in range(2):
                    nc.sync.dma_start(
                        xpad[b][cb][:, ROW0 : ROW0 + H * W],
                        x_d[b, cb * 128 : (cb + 1) * 128, :, :].rearrange(
                            "c h w -> c (h w)"
                        ),
                    )

            # wdyn slab per c-block, blocks of (dy, k, dx, o): one 6KB-elem
            # DMA per (cb, dy) keeps the modeled DMA pipe at full rate
            CHW = K * KS * O  # 3072: one (dy) block of all k
            wdyn_sl = [
                wdynp.tile([128, KS * CHW], bf16, tag=f"wds{cb}", name=f"wdyn{cb}")
                for cb in range(2)
            ]

            def wdk(cb, dy, k, lo, hi):
                off = dy * CHW + k * KS * O
                return wdyn_sl[cb][:, off + lo : off + hi]

            load_x(0)

            # MLP consts packed into one [128, 196] tensor, one DMA on the
            # ACT queue: cols 0:128 fc1wT (cb-major), 128:132 fc2aug rows,
            # 132:196 fc1b on partition row 0
            mlpc_sb = constp.tile([128, 2 * MID + K + MID], f32, tag="mlpc", name="mlpc_sb")
            nc.scalar.dma_start(mlpc_sb, mlpc_d)
            fc1wT_sb = mlpc_sb[:, 0 : 2 * MID]
            fc2aug_sb = mlpc_sb[0 : MID + 1, 2 * MID : 2 * MID + K]
            fc1b_sb = mlpc_sb[0:1, 2 * MID + K : 2 * MID + K + MID]

            # ---------------- pooling + attention ----------------------------
            # Two half-image partials per c-block, DVE and ACT in parallel.
            def pool_sample(b):
                pooled = []
                HHW = H * W // 2
                for cb in range(2):
                    pp = constp.tile([128, 2], f32, tag=f"pp{b}{cb}", name=f"pp{b}{cb}")
                    nc.vector.reduce_sum(
                        pp[:, 0:1], xpad[b][cb][:, ROW0 : ROW0 + HHW], AX.X
                    )
                    nc.scalar.activation(
                        act_dummy, xpad[b][cb][:, ROW0 + HHW : ROW0 + H * W],
                        ACTF.Copy, accum_out=pp[:, 1:2],
                    )
                    p = constp.tile([128, 1], f32, tag=f"pool{b}{cb}", name=f"pooled{b}{cb}")
                    nc.vector.reduce_sum(p, pp, AX.X)
                    pooled.append(p)
                return pooled

            def attn_mlp(b, pooled, first_dep):
                hid_ps = smallps.tile([MID, 1], f32, tag="conv", name=f"hid_ps{b}")
                first_mm = None
                for cb in range(2):
                    mm = nc.tensor.matmul(
                        hid_ps,
                        fc1wT_sb[:, cb * MID : (cb + 1) * MID],
                        pooled[cb],
                        start=(cb == 0),
                        stop=False,
                    )
                    if first_mm is None:
                        first_mm = mm
                        if first_dep is not None:
                            add_dep_helper(mm.ins, first_dep.ins, sync=False,
                                           reason="PE order for MLP")
                nc.tensor.matmul(hid_ps, fc1b_sb, ones_sb[:, 0:1], start=False, stop=True)

                hid_sb = constp.tile([MID + 1, 1], f32, tag=f"hid{b}", name=f"hid_sb{b}")
                nc.gpsimd.memset(hid_sb[MID : MID + 1, :], 1.0)
                nc.scalar.activation(hid_sb[0:MID, :], hid_ps, ACTF.Relu)

                lg_ps = smallps.tile([1, K], f32, tag="conv", name=f"lg_ps{b}")
                nc.tensor.matmul(lg_ps, hid_sb, fc2aug_sb, start=True, stop=True)

                # softmax, unnormalized: broadcast exp(logits) immediately; the
                # 1/sum normalization is folded into the PSUM->SBUF copy scale
                # and the border-correction subtract.
                ex = constp.tile([1, K], f32, tag=f"ex{b}", name=f"ex{b}")
                sm = constp.tile([1, 1], f32, tag=f"sm{b}", name=f"sm{b}")
                nc.scalar.activation(ex, lg_ps, ACTF.Exp, accum_out=sm)
                ex_bc = constp.tile([128, K], f32, tag=f"exbc{b}", name=f"ex_bc{b}")
                nc.gpsimd.partition_broadcast(ex_bc, ex)

                rcn = constp.tile([1, 2], f32, tag=f"rcn{b}", name=f"rcn{b}")
                nc.vector.reciprocal(rcn[:, 0:1], sm)
                nc.vector.tensor_scalar_mul(rcn[:, 1:2], rcn[:, 0:1], -1.0)
                rc_bc = constp.tile([128, 2], f32, tag=f"rcbc{b}", name=f"rc_bc{b}")
                nc.gpsimd.partition_broadcast(rc_bc, rcn)
                return ex_bc, rc_bc

            pooled0 = pool_sample(0)
            ex_bc0, rc_bc0 = attn_mlp(0, pooled0, None)

            # ---------------- wdyn loads: (cb, dy) interleaved ---------------
            for cb, dy in PASSES:
                nc.sync.dma_start(
                    wdyn_sl[cb][:, dy * CHW : (dy + 1) * CHW], wd_d[dy, cb]
                )

            # ---------------- weight mixing: bf16 ts/tt trees on DVE ---------
            # Chunk = (cb, dy) in arrival order.  Tree per chunk:
            #   wm = e0*w0; s1 = e1*w1; wm += s1; s1 = e2*w2; s2 = e3*w3;
            #   s1 += s2; wm += s1     (4x tensor_scalar, 2x tensor_tensor)
            mix_s1 = constp.tile([128, KS * O], bf16, tag="mixs1", name="mix_s1")
            mix_s2 = constp.tile([128, KS * O], bf16, tag="mixs2", name="mix_s2")
            mix_s3 = constp.tile([128, KS * O], bf16, tag="mixs3", name="mix_s3")
            wmix = [[None, None] for _ in range(NB)]
            mix_last = [None]

            def mix_chunk(wm, cb, dy, ex_bc, lo, hi, act_assist):
                wmh = wm[:, dy * KS * O + lo : dy * KS * O + hi]
                sl = mix_s1[:, 0 : hi - lo]
                s2 = mix_s2[:, 0 : hi - lo]
                s3 = mix_s3[:, 0 : hi - lo]
                first = nc.vector.tensor_scalar_mul(
                    wmh, wdk(cb, dy, 0, lo, hi), ex_bc[:, 0:1]
                )
                if mix_last[0] is not None:
                    add_dep_helper(first.ins, mix_last[0].ins, sync=False,
                                   reason="mix chunk order")
                nc.vector.tensor_scalar_mul(sl, wdk(cb, dy, 1, lo, hi), ex_bc[:, 1:2])
                nc.vector.tensor_tensor(wmh, wmh, sl, op=ALU.add)
                if act_assist:
                    nc.scalar.activation(s2, wdk(cb, dy, 2, lo, hi), ACTF.Copy,
                                         scale=ex_bc[:, 2:3])
                    nc.scalar.activation(s3, wdk(cb, dy, 3, lo, hi), ACTF.Copy,
                                         scale=ex_bc[:, 3:4])
                else:
                    nc.vector.tensor_scalar_mul(s2, wdk(cb, dy, 2, lo, hi), ex_bc[:, 2:3])
                    nc.vector.tensor_scalar_mul(s3, wdk(cb, dy, 3, lo, hi), ex_bc[:, 3:4])
                nc.vector.tensor_tensor(s2, s2, s3, op=ALU.add)
                mix_last[0] = nc.vector.tensor_tensor(wmh, wmh, s2, op=ALU.add)

            def mix_sample(b, ex_bc, fine_first, act_assist):
                for cb in range(2):
                    if wmix[b][cb] is None:
                        wmix[b][cb] = wmixp.tile(
                            [128, NPOS * O], bf16, tag=f"wm{b}{cb}", name=f"wmix{b}{cb}"
                        )
                for pi, (cb, dy) in enumerate(PASSES):
                    wm = wmix[b][cb]
                    if pi == 0 and fine_first:
                        for dx in range(KS):
                            mix_chunk(wm, cb, dy, ex_bc, dx * O, (dx + 1) * O,
                                      act_assist)
                    else:
                        mix_chunk(wm, cb, dy, ex_bc, 0, KS * O, act_assist)

            mix_sample(0, ex_bc0, True, True)

            # x[1] queues behind wdyn on the bulk DMA stream
            load_x(1)

            # ---------------- conv ------------------------------------------
            def wsl(b, cb, pos, ob):
                off = pos * O + ob * 128
                return wmix[b][cb][:, off : off + 128]

            TILES = [(ob, rg) for ob in range(2) for rg in range(8)]
            GROUPS = [TILES[0:5], TILES[5:10], TILES[10:15], TILES[15:16]]

            def corr_block(b, ob, rc_bc):
                corr = corrps.tile([128, 128], f32, tag="corr", name=f"corr{b}{ob}")
                for side, dxv in ((0, 0), (1, 2)):
                    i = 0
                    for cb in range(2):
                        for dy in range(KS):
                            s = dy * W + (0 if side == 0 else ROW0)
                            rhs = xpad[b][cb][:, s : s + (H - 1) * W + 1 : W]
                            nc.tensor.matmul(
                                corr[:, side * 64 : side * 64 + 64],
                                wsl(b, cb, dy * KS + dxv, ob),
                                rhs,
                                start=(i == 0),
                                stop=(i == 5),
                            )
                            i += 1
                return corr

            def conv_sample(b, ex_bc, rc_bc):
                corr = {}
                for group in GROUPS:
                    cps = {}
                    for ob, rg in group:
                        cps[(ob, rg)] = convps.tile(
                            [128, 512], f32, tag="conv", name=f"cps{b}{ob}{rg}"
                        )
                    last_mm = None

                    for pi, (cb, dy) in enumerate(PASSES):
                        final = pi == len(PASSES) - 1
                        if final:
                            order = [
                                (ob, rg, pos)
                                for ob, rg in group
                                for pos in range(dy * KS, (dy + 1) * KS)
                            ]
                        else:
                            order = [
                                (ob, rg, pos)
                                for pos in range(dy * KS, (dy + 1) * KS)
                                for ob, rg in group
                            ]
                        for ob, rg, pos in order:
                            ddy, dx = divmod(pos, 3)
                            s = (rg * 8 + ddy) * W + dx
                            last_mm = nc.tensor.matmul(
                                cps[(ob, rg)],
                                wsl(b, cb, pos, ob),
                                xpad[b][cb][:, s : s + 512],
                                start=(pi == 0 and pos == dy * KS),
                                stop=(final and pos == (dy + 1) * KS - 1),
                            )

                    # border corrections once per ob (needs all 6 wmix chunks)
                    for ob in sorted({ob for ob, _ in group}):
                        if (b, ob) not in corr:
                            corr[(b, ob)] = corr_block(b, ob, rc_bc)

                    for ob, rg in group:
                        y0 = rg * 8
                        osb = osbp.tile([128, 512], f32, tag="osb", name=f"osb{b}{ob}{rg}")
                        # PSUM->SBUF copy applies the softmax 1/sum scale
                        nc.scalar.activation(osb, cps[(ob, rg)], ACTF.Copy,
                                             scale=rc_bc[:, 0:1])
                        ov = osb.rearrange("m (y x) -> m y x", x=W)[:, :, 0 : W : W - 1]
                        cv = corr[(b, ob)].rearrange("m (s y) -> m y s", s=2)[:, y0 : y0 + 8, :]
                        # ov -= corr/sum  ==  ov += corr * (-1/sum)
                        nc.vector.scalar_tensor_tensor(
                            ov, cv, rc_bc[:, 1:2], ov, op0=ALU.mult, op1=ALU.add
                        )
                        nc.sync.dma_start(
                            out_d[b, ob * 128 : (ob + 1) * 128, y0 : y0 + 8, :],
                            osb.rearrange("m (y x) -> m y x", x=W),
                        )
                    yield last_mm

            g0 = conv_sample(0, ex_bc0, rc_bc0)
            next(g0)  # G1
            g2_last = next(g0)  # G2

            # sample-1 attention between b0's conv groups: pinned behind G2 on
            # the PE stream; with 6 conv banks a slightly-delayed copy no
            # longer stalls the next group
            pooled1 = pool_sample(1)
            ex_bc1, rc_bc1 = attn_mlp(1, pooled1, g2_last)
            mix_sample(1, ex_bc1, False, False)

            for _ in g0:  # G3, G4
                pass
            for _ in conv_sample(1, ex_bc1, rc_bc1):
                pass

    nc.compile()
    return nc


def get_nc():
    if "nc" not in _CACHE:
        _CACHE["nc"] = _build_nc()
    return _CACHE["nc"]


def prep_inputs(x, w_dyn, fc1_w, fc1_b, fc2_w, fc2_b):
    """Host-side layout prep + batch sharding -> per-core input maps."""
    import ml_dtypes

    bf16 = ml_dtypes.bfloat16
    # [dy, cb, c', (k dx o)] so each (cb, dy) chunk is one DMA with 6KB
    # contiguous per partition
    w_dynT = np.ascontiguousarray(
        np.transpose(np.asarray(w_dyn, np.float32), (3, 2, 0, 4, 1))
        .reshape(KS, 2, 128, K, KS, O)
        .reshape(KS, 2, 128, K * KS * O)
    ).astype(bf16)
    fc1wT = np.asarray(fc1_w, np.float32).T / float(H * W)      # [C, MID]
    fc2aug = (
        np.vstack([np.asarray(fc2_w, np.float32).T, np.asarray(fc2_b, np.float32)[None, :]])
        * INV_DELTA
    )                                                            # [MID+1, K]
    mlpc = np.zeros((128, 2 * MID + K + MID), np.float32)
    mlpc[:, 0:MID] = fc1wT[0:128]
    mlpc[:, MID : 2 * MID] = fc1wT[128:256]
    mlpc[0 : MID + 1, 2 * MID : 2 * MID + K] = fc2aug
    mlpc[0, 2 * MID + K :] = np.asarray(fc1_b, np.float32)
    x = np.asarray(x, np.float32).astype(bf16)
    in_maps = []
    for core in range(NCORES):
        in_maps.append(
            {
                "x": np.ascontiguousarray(x[core * NB : (core + 1) * NB]),
                "wdynT": w_dynT,
                "mlpc": mlpc,
            }
        )
    return in_maps


def kernel(x, w_dyn, fc1_w, fc1_b, fc2_w, fc2_b):
    from concourse.bass_utils import run_bass_kernel_spmd

    nc = get_nc()
    in_maps = prep_inputs(x, w_dyn, fc1_w, fc1_b, fc2_w, fc2_b)
    res = run_bass_kernel_spmd(nc, in_maps, core_ids=list(range(NCORES)))
    return np.concatenate([r["out"] for r in res.results], axis=0)


# revision 21
# speedup vs baseline: 1.1785x; 1.1785x over previous
"""Trainium2 Bass kernel for nn_ConvDY2d (dynamic-weight 3x3 conv, CondConv-style).

Reference computation (B=16, C=O=256, H=W=64, K=4 mixing kernels):
  attn  = softmax(MLP(global_avg_pool(x)) / 30)            # [B, 4]
  w_mix = einsum('bk,koihw->boihw', attn, w_dyn)           # per-sample 3x3 conv kernel
  out[b] = conv2d(x[b], w_mix[b], padding=1)

Strategy: data-parallel over batch, 2 samples per NeuronCore across 8 cores.
Per core, the conv is an implicit GEMM: for each (out-channel block, 8-row
group) a [128, 512] PSUM tile accumulates 18 matmuls (2 c-blocks x 9 taps)
whose rhs are contiguous 512-element slices of a row-padded input image
([128c, 4226]).  Column wrap-around at row edges is fixed up afterwards by
subtracting border corrections computed with 12 strided-rhs matmuls per
output block.

Startup-latency schedule (the modeled DMA pipe is the whole game: packets
flow at ~230GB/s effective and a DMA's data is only "visible" ~1us after
its last packet, so everything is ordered around byte arrival):
  - x[b0] first as one 1MB DMA per c-block; pooling runs as two half-image
    partials per c-block, split across DVE and ACT.
  - MLP consts go on the DVE DMA queue so they bypass the bulk stream.
  - wdyn is loaded in (dy-row, c-block) interleaved order and the conv
    accumulation passes consume in exactly that order, so the matmul
    stream chases the arrivals with ~1us of margin everywhere.
  - weight mixing uses tensor_scalar (4x DVE mode) + tensor_tensor (2x)
    trees in bf16; the very first chunk is mixed per-tap ([128,256]) to
    cut first-matmul latency.
"""

import sys

if "/opt/trn_rl_repo" not in sys.path:
    sys.path.insert(0, "/opt/trn_rl_repo")

import numpy as np

B, C, H, W = 16, 256, 64, 64
O, K, KS = 256, 4, 3
MID = C // 4
INV_DELTA = 1.0 / 30.0
NCORES = 8
NB = B // NCORES            # samples per core
NPOS = KS * KS              # 9 taps
FPAD = 1 + 66 * W + 1       # padded image free size: 4226
ROW0 = 65                   # flat offset of input row 0 (= 1 + 1*64)

_CACHE = {}


def _build_nc():
    import concourse.bacc as bacc
    import concourse.tile as tile
    from concourse import mybir
    from concourse.tile_rust import add_dep_helper

    f32 = mybir.dt.float32
    bf16 = mybir.dt.bfloat16
    AX = mybir.AxisListType
    ALU = mybir.AluOpType
    ACTF = mybir.ActivationFunctionType

    nc = bacc.Bacc(target_bir_lowering=False, debug=False)

    x_d = nc.dram_tensor("x", [NB, C, H, W], bf16, kind="ExternalInput").ap()
    wd_d = nc.dram_tensor("wdynT", [KS, 2, 128, K * KS * O], bf16, kind="ExternalInput").ap()
    mlpc_d = nc.dram_tensor("mlpc", [128, 2 * MID + K + MID], f32, kind="ExternalInput").ap()
    out_d = nc.dram_tensor("out", [NB, O, H, W], f32, kind="ExternalOutput").ap()

    # conv accumulation passes in wdyn-arrival order
    PASSES = [(0, 0), (1, 0), (0, 1), (1, 1), (0, 2), (1, 2)]  # (cb, dy)

    with tile.TileContext(nc) as tc:
        with (
            tc.tile_pool(name="consts", bufs=1) as constp,
            tc.tile_pool(name="wdyn", bufs=1) as wdynp,
            tc.tile_pool(name="wmix", bufs=1) as wmixp,
            tc.tile_pool(name="xpad", bufs=1) as xpadp,
            tc.tile_pool(name="osb", bufs=6) as osbp,
            tc.tile_pool(name="convps", bufs=5, space="PSUM") as convps,
            tc.tile_pool(name="corrps", bufs=2, space="PSUM") as corrps,
            tc.tile_pool(name="smallps", bufs=1, space="PSUM") as smallps,
        ):
            ones_sb = constp.tile([1, 128], f32, tag="ones", name="ones_sb")
            nc.gpsimd.memset(ones_sb, 1.0)
            act_dummy = constp.tile([128, 32 * W], bf16, tag="actdum", name="act_dummy")

            # xpad tiles + pad memsets for both samples up front (gpsimd idle)
            xpad = [[None, None] for _ in range(NB)]
            for b in range(NB):
                for cb in range(2):
                    t = xpadp.tile([128, FPAD], bf16, tag=f"xpad{b}{cb}", name=f"xpad{b}{cb}")
                    nc.gpsimd.memset(t[:, 0:ROW0], 0.0)
                    nc.gpsimd.memset(t[:, ROW0 + H * W : FPAD], 0.0)
                    xpad[b][cb] = t

            def load_x(b):
                # one DMA per c-block: 128 descriptors x 8KB, minimal
                # packet/semaphore overhead in the modeled DMA pipe
                for cb in range(2):
                    nc.sync.dma_start(
                        xpad[b][cb][:, ROW0 : ROW0 + H * W],
                        x_d[b, cb * 128 : (cb + 1) * 128, :, :].rearrange(
                            "c h w -> c (h w)"
                        ),
                    )

            # wdyn slab per c-block, blocks of (dy, k, dx, o): one 6KB-elem
            # DMA per (cb, dy) keeps the modeled DMA pipe at full rate
            CHW = K * KS * O  # 3072: one (dy) block of all k
            wdyn_sl = [
                wdynp.tile([128, KS * CHW], bf16, tag=f"wds{cb}", name=f"wdyn{cb}")
                for cb in range(2)
            ]

            def wdk(cb, dy, k, lo, hi):
                off = dy * CHW + k * KS * O
                return wdyn_sl[cb][:, off + lo : off + hi]

            load_x(0)

            # MLP consts packed into one [128, 196] tensor, one DMA on the
            # ACT queue: cols 0:128 fc1wT (cb-major), 128:132 fc2aug rows,
            # 132:196 fc1b on partition row 0
            mlpc_sb = constp.tile([128, 2 * MID + K + MID], f32, tag="mlpc", name="mlpc_sb")
            nc.scalar.dma_start(mlpc_sb, mlpc_d)
            fc1wT_sb = mlpc_sb[:, 0 : 2 * MID]
            fc2aug_sb = mlpc_sb[0 : MID + 1, 2 * MID : 2 * MID + K]
            fc1b_sb = mlpc_sb[0:1, 2 * MID + K : 2 * MID + K + MID]

            # ---------------- pooling + attention ----------------------------
            # Two half-image partials per c-block, DVE and ACT in parallel.
            def pool_sample(b):
                pooled = []
                HHW = H * W // 2
                for cb in range(2):
                    pp = constp.tile([128, 2], f32, tag=f"pp{b}{cb}", name=f"pp{b}{cb}")
                    nc.vector.reduce_sum(
                        pp[:, 0:1], xpad[b][cb][:, ROW0 : ROW0 + HHW], AX.X
                    )
                    nc.scalar.activation(
                        act_dummy, xpad[b][cb][:, ROW0 + HHW : ROW0 + H * W],
                        ACTF.Copy, accum_out=pp[:, 1:2],
                    )
                    p = constp.tile([128, 1], f32, tag=f"pool{b}{cb}", name=f"pooled{b}{cb}")
                    nc.vector.reduce_sum(p, pp, AX.X)
                    pooled.append(p)
                return pooled

            def attn_mlp(b, pooled, first_dep):
                hid_ps = smallps.tile([MID, 1], f32, tag="small", name=f"hid_ps{b}")
                first_mm = None
                for cb in range(2):
                    mm = nc.tensor.matmul(
                        hid_ps,
                        fc1wT_sb[:, cb * MID : (cb + 1) * MID],
                        pooled[cb],
                        start=(cb == 0),
                        stop=False,
                    )
                    if first_mm is None:
                        first_mm = mm
                        if first_dep is not None:
                            add_dep_helper(mm.ins, first_dep.ins, sync=False,
                                           reason="PE order for MLP")
                nc.tensor.matmul(hid_ps, fc1b_sb, ones_sb[:, 0:1], start=False, stop=True)

                hid_sb = constp.tile([MID + 1, 1], f32, tag=f"hid{b}", name=f"hid_sb{b}")
                nc.gpsimd.memset(hid_sb[MID : MID + 1, :], 1.0)
                nc.scalar.activation(hid_sb[0:MID, :], hid_ps, ACTF.Relu)

                lg_ps = smallps.tile([1, K], f32, tag="small", name=f"lg_ps{b}")
                nc.tensor.matmul(lg_ps, hid_sb, fc2aug_sb, start=True, stop=True)

                # softmax, unnormalized: broadcast exp(logits) immediately; the
                # 1/sum normalization is folded into the PSUM->SBUF copy scale
                # and the border-correction subtract.
                ex = constp.tile([1, K], f32, tag=f"ex{b}", name=f"ex{b}")
                sm = constp.tile([1, 1], f32, tag=f"sm{b}", name=f"sm{b}")
                nc.scalar.activation(ex, lg_ps, ACTF.Exp, accum_out=sm)
                ex_bc = constp.tile([128, K], f32, tag=f"exbc{b}", name=f"ex_bc{b}")
                nc.gpsimd.partition_broadcast(ex_bc, ex)

                rcn = constp.tile([1, 2], f32, tag=f"rcn{b}", name=f"rcn{b}")
                nc.vector.reciprocal(rcn[:, 0:1], sm)
                nc.vector.tensor_scalar_mul(rcn[:, 1:2], rcn[:, 0:1], -1.0)
                rc_bc = constp.tile([128, 2], f32, tag=f"rcbc{b}", name=f"rc_bc{b}")
                nc.gpsimd.partition_broadcast(rc_bc, rcn)
                return ex_bc, rc_bc

            pooled0 = pool_sample(0)
            ex_bc0, rc_bc0 = attn_mlp(0, pooled0, None)

            # ---------------- wdyn loads: (cb, dy) interleaved ---------------
            for cb, dy in PASSES:
                nc.sync.dma_start(
                    wdyn_sl[cb][:, dy * CHW : (dy + 1) * CHW], wd_d[dy, cb]
                )

            # ---------------- weight mixing: bf16 ts/tt trees on DVE ---------
            # Chunk = (cb, dy) in arrival order.  Tree per chunk:
            #   wm = e0*w0; s1 = e1*w1; wm += s1; s1 = e2*w2; s2 = e3*w3;
            #   s1 += s2; wm += s1     (4x tensor_scalar, 2x tensor_tensor)
            mix_s1 = constp.tile([128, KS * O], bf16, tag="mixs1", name="mix_s1")
            mix_s2 = constp.tile([128, KS * O], bf16, tag="mixs2", name="mix_s2")
            mix_s3 = constp.tile([128, KS * O], bf16, tag="mixs3", name="mix_s3")
            wmix = [[None, None] for _ in range(NB)]
            mix_last = [None]

            def mix_chunk(wm, cb, dy, ex_bc, lo, hi, act_assist):
                wmh = wm[:, dy * KS * O + lo : dy * KS * O + hi]
                sl = mix_s1[:, 0 : hi - lo]
                s2 = mix_s2[:, 0 : hi - lo]
                s3 = mix_s3[:, 0 : hi - lo]
                first = nc.vector.tensor_scalar_mul(
                    wmh, wdk(cb, dy, 0, lo, hi), ex_bc[:, 0:1]
                )
                if mix_last[0] is not None:
                    add_dep_helper(first.ins, mix_last[0].ins, sync=False,
                                   reason="mix chunk order")
                nc.vector.tensor_scalar_mul(sl, wdk(cb, dy, 1, lo, hi), ex_bc[:, 1:2])
                nc.vector.tensor_tensor(wmh, wmh, sl, op=ALU.add)
                if act_assist:
                    nc.scalar.activation(s2, wdk(cb, dy, 2, lo, hi), ACTF.Copy,
                                         scale=ex_bc[:, 2:3])
                    nc.scalar.activation(s3, wdk(cb, dy, 3, lo, hi), ACTF.Copy,
                                         scale=ex_bc[:, 3:4])
                else:
                    nc.vector.tensor_scalar_mul(s2, wdk(cb, dy, 2, lo, hi), ex_bc[:, 2:3])
                    nc.vector.tensor_scalar_mul(s3, wdk(cb, dy, 3, lo, hi), ex_bc[:, 3:4])
                nc.vector.tensor_tensor(s2, s2, s3, op=ALU.add)
                mix_last[0] = nc.vector.tensor_tensor(wmh, wmh, s2, op=ALU.add)

            def mix_sample(b, ex_bc, fine_first, act_assist):
                for cb in range(2):
                    if wmix[b][cb] is None:
                        wmix[b][cb] = wmixp.tile(
                            [128, NPOS * O], bf16, tag=f"wm{b}{cb}", name=f"wmix{b}{cb}"
                        )
                for pi, (cb, dy) in enumerate(PASSES):
                    wm = wmix[b][cb]
                    if pi == 0 and fine_first:
                        for dx in range(KS):
                            mix_chunk(wm, cb, dy, ex_bc, dx * O, (dx + 1) * O,
                                      act_assist)
                    else:
                        mix_chunk(wm, cb, dy, ex_bc, 0, KS * O, act_assist)

            mix_sample(0, ex_bc0, True, True)

            # x[1] queues behind wdyn on the bulk DMA stream
            load_x(1)

            # ---------------- conv ------------------------------------------
            def wsl(b, cb, pos, ob):
                off = pos * O + ob * 128
                return wmix[b][cb][:, off : off + 128]

            TILES = [(ob, rg) for ob in range(2) for rg in range(8)]
            GROUPS = [TILES[0:5], TILES[5:10], TILES[10:15], TILES[15:16]]

            def corr_block(b, ob, rc_bc):
                corr = corrps.tile([128, 128], f32, tag="corr", name=f"corr{b}{ob}")
                for side, dxv in ((0, 0), (1, 2)):
                    i = 0
                    for cb in range(2):
                        for dy in range(KS):
                            s = dy * W + (0 if side == 0 else ROW0)
                            rhs = xpad[b][cb][:, s : s + (H - 1) * W + 1 : W]
                            nc.tensor.matmul(
                                corr[:, side * 64 : side * 64 + 64],
                                wsl(b, cb, dy * KS + dxv, ob),
                                rhs,
                                start=(i == 0),
                                stop=(i == 5),
                            )
                            i += 1
                return corr

            def conv_sample(b, ex_bc, rc_bc):
                corr = {}
                for group in GROUPS:
                    cps = {}
                    for ob, rg in group:
                        cps[(ob, rg)] = convps.tile(
                            [128, 512], f32, tag="conv", name=f"cps{b}{ob}{rg}"
                        )
                    last_mm = None

                    for pi, (cb, dy) in enumerate(PASSES):
                        final = pi == len(PASSES) - 1
                        if final:
                            order = [
                                (ob, rg, pos)
                                for ob, rg in group
                                for pos in range(dy * KS, (dy + 1) * KS)
                            ]
                        else:
                            order = [
                                (ob, rg, pos)
                                for pos in range(dy * KS, (dy + 1) * KS)
                                for ob, rg in group
                            ]
                        for ob, rg, pos in order:
                            ddy, dx = divmod(pos, 3)
                            s = (rg * 8 + ddy) * W + dx
                            last_mm = nc.tensor.matmul(
                                cps[(ob, rg)],
                                wsl(b, cb, pos, ob),
                                xpad[b][cb][:, s : s + 512],
                                start=(pi == 0 and pos == dy * KS),
                                stop=(final and pos == (dy + 1) * KS - 1),
                            )

                    # border corrections once per ob (needs all 6 wmix chunks)
                    for ob in sorted({ob for ob, _ in group}):
                        if (b, ob) not in corr:
                            corr[(b, ob)] = corr_block(b, ob, rc_bc)

                    for ob, rg in group:
                        y0 = rg * 8
                        osb = osbp.tile([128, 512], f32, tag="osb", name=f"osb{b}{ob}{rg}")
                        # PSUM->SBUF copy applies the softmax 1/sum scale
                        nc.scalar.activation(osb, cps[(ob, rg)], ACTF.Copy,
                                             scale=rc_bc[:, 0:1])
                        ov = osb.rearrange("m (y x) -> m y x", x=W)[:, :, 0 : W : W - 1]
                        cv = corr[(b, ob)].rearrange("m (s y) -> m y s", s=2)[:, y0 : y0 + 8, :]
                        # ov -= corr/sum  ==  ov += corr * (-1/sum)
                        nc.vector.scalar_tensor_tensor(
                            ov, cv, rc_bc[:, 1:2], ov, op0=ALU.mult, op1=ALU.add
                        )
                        nc.sync.dma_start(
                            out_d[b, ob * 128 : (ob + 1) * 128, y0 : y0 + 8, :],
                            osb.rearrange("m (y x) -> m y x", x=W),
                        )
                    yield last_mm

            g0 = conv_sample(0, ex_bc0, rc_bc0)
            next(g0)  # G1
            g2_last = next(g0)  # G2

            # sample-1 attention between b0's conv groups: pinned behind G2 on
            # the PE stream; with 6 conv banks a slightly-delayed copy no
            # longer stalls the next group
            pooled1 = pool_sample(1)
            ex_bc1, rc_bc1 = attn_mlp(1, pooled1, g2_last)
            mix_sample(1, ex_bc1, False, False)

            for _ in g0:  # G3, G4
                pass
            for _ in conv_sample(1, ex_bc1, rc_bc1):
                pass

    nc.compile()
    return nc


def get_nc():
    if "nc" not in _CACHE:
        _CACHE["nc"] = _build_nc()
    return _CACHE["nc"]


def prep_inputs(x, w_dyn, fc1_w, fc1_b, fc2_w, fc2_b):
    """Host-side layout prep + batch sharding -> per-core input maps."""
    import ml_dtypes

    bf16 = ml_dtypes.bfloat16
    # [dy, cb, c', (k dx o)] so each (cb, dy) chunk is one DMA with 6KB
    # contiguous per partition
    w_dynT = np.ascontiguousarray(
        np.transpose(np.asarray(w_dyn, np.float32), (3, 2, 0, 4, 1))
        .reshape(KS, 2, 128, K, KS, O)
        .reshape(KS, 2, 128, K * KS * O)
    ).astype(bf16)
    fc1wT = np.asarray(fc1_w, np.float32).T / float(H * W)      # [C, MID]
    fc2aug = (
        np.vstack([np.asarray(fc2_w, np.float32).T, np.asarray(fc2_b, np.float32)[None, :]])
        * INV_DELTA
    )                                                            # [MID+1, K]
    mlpc = np.zeros((128, 2 * MID + K + MID), np.float32)
    mlpc[:, 0:MID] = fc1wT[0:128]
    mlpc[:, MID : 2 * MID] = fc1wT[128:256]
    mlpc[0 : MID + 1, 2 * MID : 2 * MID + K] = fc2aug
    mlpc[0, 2 * MID + K :] = np.asarray(fc1_b, np.float32)
    x = np.asarray(x, np.float32).astype(bf16)
    in_maps = []
    for core in range(NCORES):
        in_maps.append(
            {
                "x": np.ascontiguousarray(x[core * NB : (core + 1) * NB]),
                "wdynT": w_dynT,
                "mlpc": mlpc,
            }
        )
    return in_maps


def kernel(x, w_dyn, fc1_w, fc1_b, fc2_w, fc2_b):
    from concourse.bass_utils import run_bass_kernel_spmd

    nc = get_nc()
    in_maps = prep_inputs(x, w_dyn, fc1_w, fc1_b, fc2_w, fc2_b)
    res = run_bass_kernel_spmd(nc, in_maps, core_ids=list(range(NCORES)))
    return np.concatenate([r["out"] for r in res.results], axis=0)
